# revision 17
# baseline (speedup 1.0000x reference)
"""Trainium2 Bass kernel for nn_AugmentedAttentionHead.

Math per batch b (reference):
  q = LN(x @ w_q) ; k = LN(x @ w_k) ; v = x @ w_v          [T, H]
  sim = q @ k^T                                            [T, T]
  alpha = softplus(q[1:] @ w_alpha + b_alpha)              [N]
  sigma = sigmoid(q[1:] @ w_sigma + b_sigma)               [N, 2]
  G[p, k] = exp(-dx2/(2 sx_p^2) - dy2/(2 sy_p^2))          [N, N]
  sim[1:, 1:] += alpha * G
  out = softmax(sim / 8) @ v                               [T, H]

Strategy: data-parallel over batch (8 batches/core on 8 cores).
On-chip layout: softmax computed TRANSPOSED ([k-token partitions, q-token
free]) so the exp() output can feed the attn @ v matmul as the stationary
operand directly (contraction over k needs k on partitions), avoiding any
transpose/evacuation of the [T, T] attention matrix.

The Gaussian bias is built in log space: for each query token t the row
factors Y[t, j] = -(py_t - j)^2 b'_t / ... and X[t, i] plus ln(alpha'_t) are
assembled as a [T, 49] tile, transposed once (PE), and then ONE f32r matmul
per k-chunk with a constant 0/1 selector matrix produces
  L'[k, q] = ln(alpha'_q) - dy2(k,q)/(2 sy_q^2) - dx2(k,q)/(2 sx_q^2)
exp(L') = alpha' * G is injected into the sim PSUM via an identity matmul.

x is passed host-pre-transposed ([E, T] per batch) so the QKV projection can
contract over E with x^T chunks as the stationary operand, producing q,k,v in
natural layout (layernorm then reduces over the free axis, which is cheap).
"""

import numpy as np
from contextlib import ExitStack

B, T, E, H = 64, 577, 768, 64
GRID = 24
N = T - 1  # 576 patches
EPS = 1e-5
NCORES = 8
BPC = B // NCORES  # batches per core

TSZ = [128, 128, 128, 128, 65]  # T split into 5 chunks
TOFF = [0, 128, 256, 384, 512]
NCH = 5
FW = 49  # factor width: 24 (y) + 1 (ln alpha) + 24 (x)


def _host_consts(w_q, w_k, w_v, w_sigma, w_alpha):
    """Pack host-side constant tensors."""
    # w_ext: [6, 128, 256] — per E-chunk: [w_q | w_k | w_v | zero pad]
    w_ext = np.zeros((6, 128, 256), np.float32)
    wqkv = np.concatenate([w_q, w_k, w_v], axis=1)  # [768, 192]
    for j in range(6):
        w_ext[j, :, :192] = wqkv[j * 128:(j + 1) * 128]

    # w_sa: [64, 4] = [8*w_sigma | 8*w_alpha | pad]  (q^T is pre-scaled by
    # rstd/8 for the sim matmul; the *8 undoes that for the sigma/alpha MLP)
    w_sa = np.zeros((64, 4), np.float32)
    w_sa[:, 0:2] = 8.0 * w_sigma
    w_sa[:, 2] = 8.0 * w_alpha[:, 0]

    # dxy2s: [128, 5*49] — per token row r, chunk c (token t = 128c + r,
    # patch p = t - 1): cols 0:24 = -0.5*(py - j)^2, col 24 = 0,
    # cols 25:49 = -0.5*(px - i)^2. Token 0 row: zeros.
    dxy2s = np.zeros((128, NCH * FW), np.float32)
    for c in range(NCH):
        for r in range(TSZ[c]):
            t = TOFF[c] + r
            if t == 0:
                continue
            p = t - 1
            py, px = p // GRID, p % GRID
            j = np.arange(GRID, dtype=np.float32)
            dxy2s[r, c * FW:c * FW + 24] = -0.5 * (py - j) ** 2
            dxy2s[r, c * FW + 25:c * FW + 49] = -0.5 * (px - j) ** 2

    # SEL: [49, 5*128] — selector for k-token t = 128c + r (patch p = t-1):
    # rows 0:24 one-hot at p//24, row 24 = 1 (ln alpha), rows 25:49 one-hot
    # at p%24. k-token 0: zero column (row killed after exp).
    sel = np.zeros((FW, NCH * 128), np.float32)
    for c in range(NCH):
        for r in range(TSZ[c]):
            t = TOFF[c] + r
            if t == 0:
                continue
            p = t - 1
            sel[p // GRID, c * 128 + r] = 1.0
            sel[24, c * 128 + r] = 1.0
            sel[25 + p % GRID, c * 128 + r] = 1.0

    ident = np.eye(128, dtype=np.float32)
    return w_ext, w_sa, dxy2s, sel, ident


def _trace(nc, tc, ctx, consts_f, gamma_beta):
    import concourse.bass as bass
    import concourse.mybir as mybir

    dt = mybir.dt
    AF = mybir.ActivationFunctionType
    OP = mybir.AluOpType
    bs0, bs1, ba0 = consts_f[:3]
    bs = (bs0, bs1)
    need_gb = gamma_beta is not None

    # ---- DRAM tensors ----
    xT_d = nc.dram_tensor("xT", [BPC, E, T], dt.float32r, kind="ExternalInput").ap()
    wext_d = nc.dram_tensor("w_ext", [6, 128, 256], dt.float32r, kind="ExternalInput").ap()
    wsa_d = nc.dram_tensor("w_sa", [64, 4], dt.float32r, kind="ExternalInput").ap()
    dxy_d = nc.dram_tensor("dxy2s", [128, NCH * FW], dt.float32, kind="ExternalInput").ap()
    sel_d = nc.dram_tensor("sel", [FW, NCH * 128], dt.float32r, kind="ExternalInput").ap()
    id_d = nc.dram_tensor("ident", [128, 128], dt.float32r, kind="ExternalInput").ap()
    onz_d = nc.dram_tensor("onz", [128, 2], dt.float32r, kind="ExternalInput").ap()
    if need_gb:
        gb_d = nc.dram_tensor("gb", [4, 64], dt.float32, kind="ExternalInput").ap()
    out_d = nc.dram_tensor("out", [BPC, T, H], dt.float32, kind="ExternalOutput").ap()

    # ---- SBUF pools ----
    cpool = ctx.enter_context(tc.tile_pool(name="consts", bufs=1))
    xpool = ctx.enter_context(tc.tile_pool(name="x", bufs=12))
    wkpool = ctx.enter_context(tc.tile_pool(name="work", bufs=2))
    qkpool = ctx.enter_context(tc.tile_pool(name="qk", bufs=10))
    vpool = ctx.enter_context(tc.tile_pool(name="v", bufs=10))
    apool = ctx.enter_context(tc.tile_pool(name="attn", bufs=10))
    spool = ctx.enter_context(tc.tile_pool(name="small", bufs=2))
    opool = ctx.enter_context(tc.tile_pool(name="outb", bufs=2))

    # ---- PSUM pools (8 banks total) ----
    ps_qkv = ctx.enter_context(tc.tile_pool(name="ps_qkv", bufs=1, space="PSUM"))
    ps_tp = ctx.enter_context(tc.tile_pool(name="ps_tp", bufs=2, space="PSUM"))
    ps_z = ctx.enter_context(tc.tile_pool(name="ps_z", bufs=1, space="PSUM"))
    ps_sm = ctx.enter_context(tc.tile_pool(name="ps_sm", bufs=1, space="PSUM"))

    # ---- load constants ----
    w_ext = cpool.tile([128, 6 * 256], dt.float32r)
    for j in range(6):
        nc.sync.dma_start(w_ext[:, j * 256:(j + 1) * 256], wext_d[j])
    w_sa = cpool.tile([64, 4], dt.float32r)
    nc.sync.dma_start(w_sa[:], wsa_d)
    dxy2s = cpool.tile([128, NCH * FW], dt.float32)
    nc.sync.dma_start(dxy2s[:], dxy_d)
    sel = cpool.tile([FW, NCH * 128], dt.float32r)
    nc.sync.dma_start(sel[:], sel_d)
    ident = cpool.tile([128, 128], dt.float32r)
    nc.sync.dma_start(ident[:], id_d)
    onz = cpool.tile([128, 2], dt.float32r)
    nc.sync.dma_start(onz[:], onz_d)
    if need_gb:
        gb = cpool.tile([4, 64], dt.float32)
        nc.sync.dma_start(gb[:], gb_d)

    _bias_cache = {}

    def fbias(val, tsz=128):
        val = float(val)
        if val == 0.0:
            return 0.0
        if val not in _bias_cache:
            bt = cpool.tile([128, 1], dt.float32, name=f"bias{len(_bias_cache)}")
            nc.vector.memset(bt[:], val)
            _bias_cache[val] = bt
        return _bias_cache[val][0:tsz, :]

    for b in range(BPC):
        # ---- load x^T (6 chunks of [128, 577]) ----
        xt = []
        for j in range(6):
            xj = xpool.tile([128, T], dt.float32r, tag="xT")
            nc.sync.dma_start(xj[:], xT_d[b, j * 128:(j + 1) * 128, :])
            xt.append(xj)

        qT = wkpool.tile([64, 578], dt.float32r, tag="qT")
        kT = wkpool.tile([64, T], dt.float32r, tag="kT")
        v_ext = [vpool.tile([128, 66], dt.float32r, tag="vext", name=f"vext{i}")
                 for i in range(NCH)]
        ptp = ps_tp.tile([128, 640], dt.float32r, tag="tp")
        ptpk = ps_tp.tile([128, 640], dt.float32r, tag="tp")

        for c in range(NCH):
            tsz, toff = TSZ[c], TOFF[c]
            # ---- QKV projection: out [tsz, 256] += xT_chunk^T @ w_ext ----
            pqkv = ps_qkv.tile([128, 256], dt.float32, tag="qkv")
            for j in range(6):
                nc.tensor.matmul(
                    pqkv[0:tsz, :], xt[j][:, toff:toff + tsz],
                    w_ext[:, j * 256:(j + 1) * 256],
                    start=(j == 0), stop=(j == 5),
                )
            # ---- LN stats (q and k, grouped) ----
            st = spool.tile([128, 12], dt.float32, tag="st")
            nc.vector.bn_stats(st[0:tsz, 0:6], pqkv[0:tsz, 0:64])
            nc.vector.bn_stats(st[0:tsz, 6:12], pqkv[0:tsz, 64:128])
            mv = spool.tile([128, 4], dt.float32, tag="mv")  # [mu_q, var_q, mu_k, var_k]
            nc.vector.bn_aggr(mv[0:tsz, 0:2], st[0:tsz, 0:6])
            nc.vector.bn_aggr(mv[0:tsz, 2:4], st[0:tsz, 6:12])
            # rstd = exp(-0.5*ln(var+eps)); q side also folds the 1/8 sim
            # scale via bias ln(1/8). (Keeps every ACT func in the ln/exp
            # table — no activation-table switches.)
            lnv = spool.tile([128, 2], dt.float32, tag="lnv")
            nc.scalar.activation(
                lnv[0:tsz, :], mv[0:tsz, :].rearrange("p (g s) -> p g s", g=2)[:, :, 1],
                AF.Ln, bias=fbias(EPS, tsz),
            )
            sc2 = spool.tile([128, 2], dt.float32, tag="sc2")
            nc.scalar.activation(sc2[0:tsz, 0:1], lnv[0:tsz, 0:1], AF.Exp,
                                 bias=fbias(float(np.log(0.125)), tsz), scale=-0.5)
            nc.scalar.activation(sc2[0:tsz, 1:2], lnv[0:tsz, 1:2], AF.Exp,
                                 scale=-0.5)
            nmr = spool.tile([128, 2], dt.float32, tag="nmr")  # -mu*scale
            mu = mv[0:tsz, :].rearrange("p (g s) -> p g s", g=2)[:, :, 0]
            nc.vector.scalar_tensor_tensor(
                nmr[0:tsz, :], mu, -1.0, sc2[0:tsz, :], OP.mult, OP.mult,
            )
            # ---- LN apply -> q_ln, k_ln (f32r) ----
            qk_ln = qkpool.tile([128, 128], dt.float32r, tag="qkln")
            nc.scalar.activation(
                qk_ln[0:tsz, 0:64], pqkv[0:tsz, 0:64], AF.Identity,
                bias=nmr[0:tsz, 0:1], scale=sc2[0:tsz, 0:1],
            )
            nc.scalar.activation(
                qk_ln[0:tsz, 64:128], pqkv[0:tsz, 64:128], AF.Identity,
                bias=nmr[0:tsz, 1:2], scale=sc2[0:tsz, 1:2],
            )
            if need_gb:
                # general gamma/beta: q_ln = q_ln*gamma + beta (gamma pre-
                # divided by 8 host-side for the q half to keep the /8 fold)
                nc.vector.tensor_mul(
                    qk_ln[0:tsz, 0:64], qk_ln[0:tsz, 0:64],
                    gb[0:1, :].partition_broadcast(tsz),
                )
                nc.vector.tensor_add(
                    qk_ln[0:tsz, 0:64], qk_ln[0:tsz, 0:64],
                    gb[1:2, :].partition_broadcast(tsz),
                )
                nc.vector.tensor_mul(
                    qk_ln[0:tsz, 64:128], qk_ln[0:tsz, 64:128],
                    gb[2:3, :].partition_broadcast(tsz),
                )
                nc.vector.tensor_add(
                    qk_ln[0:tsz, 64:128], qk_ln[0:tsz, 64:128],
                    gb[3:4, :].partition_broadcast(tsz),
                )
            # ---- v evac (+ ones column for the denominator) ----
            nc.scalar.copy(v_ext[c][0:tsz, 0:64], pqkv[0:tsz, 128:192])
            nc.scalar.copy(v_ext[c][0:tsz, 64:66], onz[0:tsz, :])
            # ---- transpose q_ln, k_ln into [64, T] ----
            # (f32r transpose needs an even moving-dim; pad the 65 tail to 66
            # — the extra column lands in the unused [T:640] psum region)
            t2 = tsz + (tsz & 1)
            nc.tensor.transpose(
                ptp[0:64, toff:toff + t2], qk_ln[0:t2, 0:64],
                ident[0:t2, 0:t2],
            )
            nc.tensor.transpose(
                ptpk[0:64, toff:toff + t2], qk_ln[0:t2, 64:128],
                ident[0:t2, 0:t2],
            )
            if c == NCH - 1:
                nc.scalar.copy(qT[:, 0:T], ptp[0:64, 0:T])
                nc.scalar.copy(qT[:, T:578], onz[0:64, 1:2])
                nc.scalar.copy(kT[:, :], ptpk[0:64, 0:T])

        # ---- sigma/alpha MLP (natural layout) + factor build ----
        psa = ps_sm.tile([128, 20], dt.float32, tag="sm")
        for c in range(NCH):
            nc.tensor.matmul(
                psa[0:TSZ[c], 4 * c:4 * c + 4], qT[:, TOFF[c]:TOFF[c] + TSZ[c]],
                w_sa[:], start=True, stop=True,
            )
        # t = exp(-(z_sigma + b_sigma)) for sx, sy (batched over chunks)
        texp = spool.tile([128, 10], dt.float32, tag="texp")
        psa4 = psa[:].rearrange("p (c f) -> p c f", f=4)
        texp3 = texp[:].rearrange("p (c f) -> p c f", f=2)
        for col in range(2):
            nc.scalar.activation(
                texp3[:, :, col], psa4[:, :, col], AF.Exp,
                bias=fbias(-bs[col]), scale=-1.0,
            )
        ab = spool.tile([128, 10], dt.float32, tag="ab")  # [a', b'] per chunk
        nc.vector.tensor_scalar_add(ab[:], texp[:], 1.0)
        nc.vector.tensor_mul(ab[:], ab[:], ab[:])
        # ln(alpha') = ln(softplus(z_alpha + b_alpha)/8); softplus has no HW
        # table here, so softplus = ln(1 + exp(z)).
        spe = spool.tile([128, 5], dt.float32, tag="spe")
        nc.scalar.activation(spe[:], psa4[:, :, 2], AF.Exp, bias=fbias(ba0))
        spl = spool.tile([128, 5], dt.float32, tag="spl")
        nc.scalar.activation(spl[:], spe[:], AF.Ln, bias=fbias(1.0))
        lna = spool.tile([128, 5], dt.float32, tag="lna")
        nc.scalar.activation(lna[:], spl[:], AF.Ln, scale=0.125)
        # Yn factors [128, 5*49] f32r: y-half, ln-alpha col, x-half
        yn = spool.tile([128, NCH * FW], dt.float32r, tag="yn")
        dxy3 = dxy2s[:].rearrange("p (c f) -> p c f", f=FW)
        yn3 = yn[:].rearrange("p (c f) -> p c f", f=FW)
        ab3 = ab[:].rearrange("p (c f) -> p c f", f=2)
        nc.vector.scalar_tensor_tensor(
            yn3[:, :, 0:24], dxy3[:, :, 0:24], 1.0,
            ab3[:, :, 1:2].broadcast_to([128, NCH, 24]), OP.mult, OP.mult,
        )
        nc.vector.scalar_tensor_tensor(
            yn3[:, :, 25:49], dxy3[:, :, 25:49], 1.0,
            ab3[:, :, 0:1].broadcast_to([128, NCH, 24]), OP.mult, OP.mult,
        )
        nc.vector.tensor_copy(yn3[:, :, 24:25], lna[:].unsqueeze(-1))

        # ---- transpose factors -> YT [49, T] ----
        pft = ps_tp.tile([128, 640], dt.float32r, tag="tp")
        for c in range(NCH):
            t2 = TSZ[c] + (TSZ[c] & 1)
            nc.tensor.transpose(
                pft[0:FW, TOFF[c]:TOFF[c] + t2],
                yn[0:t2, c * FW:(c + 1) * FW],
                ident[0:t2, 0:t2],
            )
        yT = wkpool.tile([FW, 578], dt.float32r, tag="yT")
        nc.scalar.copy(yT[:, 0:T], pft[0:FW, 0:T])
        nc.scalar.copy(yT[:, T:578], onz[0:FW, 1:2])

        # ---- main loop over k-chunks: sim^T, bias, exp ----
        attnT = [apool.tile([128, T], dt.float32r, tag="attnT", name=f"attnT{i}")
                 for i in range(NCH)]
        for c in range(NCH):
            tsz, toff = TSZ[c], TOFF[c]
            pz = ps_z.tile([128, 640], dt.float32, tag="z")
            nc.tensor.matmul(pz[0:tsz, 0:512], kT[:, toff:toff + tsz], qT[:, 0:512],
                             start=True, stop=False)
            nc.tensor.matmul(pz[0:tsz, 512:578], kT[:, toff:toff + tsz],
                             qT[:, 512:578], start=True, stop=False)
            pl = ps_tp.tile([128, 640], dt.float32, tag="tp")
            nc.tensor.matmul(pl[0:tsz, 0:512], sel[:, c * 128:c * 128 + tsz],
                             yT[:, 0:512], start=True, stop=True)
            nc.tensor.matmul(pl[0:tsz, 512:578], sel[:, c * 128:c * 128 + tsz],
                             yT[:, 512:578], start=True, stop=True)
            expl = wkpool.tile([128, 578], dt.float32r, tag="expl")
            nc.scalar.activation(expl[0:tsz, :], pl[0:tsz, 0:578], AF.Exp)
            nc.scalar.copy(expl[0:tsz, 0:1], onz[0:tsz, 1:2])  # q prefix col
            if c == 0:
                nc.scalar.copy(expl[0:1, 0:T], onz[0:1, 1:2].broadcast_to([1, T]))
            nc.tensor.matmul(pz[0:tsz, 0:512], ident[0:tsz, 0:tsz],
                             expl[0:tsz, 0:512], start=False, stop=True)
            nc.tensor.matmul(pz[0:tsz, 512:578], ident[0:tsz, 0:tsz],
                             expl[0:tsz, 512:578], start=False, stop=True)
            nc.scalar.activation(attnT[c][0:tsz, :], pz[0:tsz, 0:T], AF.Exp)

        # ---- attn @ v (+ denominator), normalize ----
        osb = opool.tile([128, 320], dt.float32, tag="osb")
        for qc in range(NCH):
            qsz, qoff = TSZ[qc], TOFF[qc]
            po = ps_sm.tile([128, 66], dt.float32, tag="sm")
            for kc in range(NCH):
                nc.tensor.matmul(
                    po[0:qsz, :], attnT[kc][0:TSZ[kc], qoff:qoff + qsz],
                    v_ext[kc][0:TSZ[kc], :], start=(kc == 0), stop=(kc == 4),
                )
            rcp = spool.tile([128, 1], dt.float32, tag="rcp")
            nc.vector.reciprocal(rcp[0:qsz, :], po[0:qsz, 64:65])
            nc.vector.tensor_scalar_mul(
                osb[0:qsz, qc * 64:(qc + 1) * 64], po[0:qsz, 0:64], rcp[0:qsz, :],
            )
        # ---- store ----
        nc.sync.dma_start(
            out_d[b, 0:512, :].rearrange("(c p) h -> p c h", p=128),
            osb[:, 0:256].rearrange("p (c h) -> p c h", h=64),
        )
        nc.sync.dma_start(out_d[b, 512:T, :], osb[0:65, 256:320])


_CACHE = {}


def _build(consts_f, need_gb):
    import concourse.tile as tile
    from concourse import bacc

    key = (consts_f, need_gb)
    if key in _CACHE:
        return _CACHE[key]
    nc = bacc.Bacc("TRN2", target_bir_lowering=False, debug=False)
    with tile.TileContext(nc) as tc, ExitStack() as ctx:
        _trace(nc, tc, ctx, consts_f, need_gb if need_gb else None)
    nc.finalize()
    _CACHE[key] = nc
    return nc


def kernel(x, w_q, w_k, w_v, q_gamma, q_beta, k_gamma, k_beta,
           w_sigma, b_sigma, w_alpha, b_alpha):
    from concourse import bass_utils

    x = np.asarray(x, np.float32)
    w_q, w_k, w_v = (np.asarray(a, np.float32) for a in (w_q, w_k, w_v))
    w_sigma = np.asarray(w_sigma, np.float32)
    w_alpha = np.asarray(w_alpha, np.float32)
    b_sigma = np.asarray(b_sigma, np.float32)
    b_alpha = np.asarray(b_alpha, np.float32)
    q_gamma, q_beta = np.asarray(q_gamma, np.float32), np.asarray(q_beta, np.float32)
    k_gamma, k_beta = np.asarray(k_gamma, np.float32), np.asarray(k_beta, np.float32)

    trivial_gb = (
        np.allclose(q_gamma, 1) and np.allclose(k_gamma, 1)
        and np.allclose(q_beta, 0) and np.allclose(k_beta, 0)
    )

    w_ext, w_sa, dxy2s, sel, ident = _host_consts(w_q, w_k, w_v, w_sigma, w_alpha)
    consts_f = (float(b_sigma[0]), float(b_sigma[1]), float(b_alpha[0]),
                0.0, 0.0, 0.0)
    nc = _build(consts_f, not trivial_gb)

    xt = np.ascontiguousarray(
        x.reshape(NCORES, BPC, T, E).transpose(0, 1, 3, 2)
    )  # [8, BPC, E, T]

    base = {
        "w_ext": w_ext, "w_sa": w_sa, "dxy2s": dxy2s, "sel": sel, "ident": ident,
        "onz": np.stack([np.ones(128), np.zeros(128)], 1).astype(np.float32),
    }
    if not trivial_gb:
        # q-gamma pre-divided by 8 to match the rstd/8 fold
        base["gb"] = np.stack([q_gamma, q_beta / 8.0, k_gamma, k_beta]).astype(np.float32)
    in_maps = [{**base, "xT": xt[c]} for c in range(NCORES)]

    res = bass_utils.run_bass_kernel_spmd(nc, in_maps, core_ids=list(range(NCORES)))
    out = np.concatenate([res.results[c]["out"] for c in range(NCORES)], axis=0)
    return out.astype(np.float32)


# revision 20
# speedup vs baseline: 1.4471x; 1.4471x over previous
"""Trainium2 Bass kernel for nn_AugmentedAttentionHead.

Math per batch b (reference):
  q = LN(x @ w_q) ; k = LN(x @ w_k) ; v = x @ w_v          [T, H]
  sim = q @ k^T ; sim[1:, 1:] += alpha * G                 [T, T]
  out = softmax(sim / 8) @ v                               [T, H]
with alpha = softplus(q[1:] @ w_alpha), sigma = sigmoid(q[1:] @ w_sigma),
G the per-query anisotropic Gaussian over the 24x24 patch grid.

Strategy: data-parallel over batch (8 batches/core on 8 cores).
Softmax is computed TRANSPOSED ([k-token partitions, q-token free]) so the
exp() output feeds the attn @ v matmul directly as the stationary operand
(contraction over k needs k on partitions) — the [T, T] attention matrix is
never transposed or evacuated.

The Gaussian bias is built in log space: per query token a [T, 50] factor
tile (24 y-terms, ln(alpha'), 24 x-terms, prefix-kill column) is built with
per-partition scalar ops, PE-transposed, and then one f32r matmul per
k-chunk against a constant 0/1 selector reconstructs
  L'[k, q] = ln(alpha'_q) - dy2(k,q)/(2 sy_q^2) - dx2(k,q)/(2 sx_q^2)
exp(L') = alpha' * G is accumulated into the sim PSUM by an identity matmul.
Prefix row/column kills are folded into the host tables (-1e30 entries).

x is passed host-pre-transposed ([E, T] per batch) so the QKV projection
contracts over E with x^T chunks stationary, producing q,k,v in natural
layout (layernorm reduces over the free axis). rstd is computed as
exp(-0.5 ln(var+eps)) so every ACT func stays in the ln/exp table set.
"""

import numpy as np
from contextlib import ExitStack

B, T, E, H = 64, 577, 768, 64
GRID = 24
EPS = 1e-5
NCORES = 8
BPC = B // NCORES

TSZ = [128, 128, 128, 128, 65]
TOFF = [0, 128, 256, 384, 512]
NCH = 5
FW = 50  # 24 y | ln-alpha | 24 x | prefix-kill
NEG = -1.0e30


def _host_consts(w_q, w_k, w_v, w_sigma, w_alpha):
    w_ext = np.zeros((6, 128, 256), np.float32)
    wqkv = np.concatenate([w_q, w_k, w_v], axis=1)
    for j in range(6):
        w_ext[j, :, :192] = wqkv[j * 128:(j + 1) * 128]

    w_sa = np.zeros((64, 4), np.float32)
    w_sa[:, 0:2] = 8.0 * w_sigma
    w_sa[:, 2] = 8.0 * w_alpha[:, 0]

    dxy2s = np.zeros((128, NCH * FW), np.float32)
    for c in range(NCH):
        for r in range(TSZ[c]):
            t = TOFF[c] + r
            o = c * FW
            dxy2s[r, o + 49] = NEG
            if t == 0:
                dxy2s[r, o:o + 24] = NEG
                dxy2s[r, o + 25:o + 49] = NEG
                continue
            p = t - 1
            py, px = p // GRID, p % GRID
            j = np.arange(GRID, dtype=np.float32)
            dxy2s[r, o:o + 24] = -0.5 * (py - j) ** 2
            dxy2s[r, o + 25:o + 49] = -0.5 * (px - j) ** 2

    sel = np.zeros((FW, NCH * 128), np.float32)
    for c in range(NCH):
        for r in range(TSZ[c]):
            t = TOFF[c] + r
            if t == 0:
                sel[49, c * 128 + r] = 1.0
                continue
            p = t - 1
            sel[p // GRID, c * 128 + r] = 1.0
            sel[24, c * 128 + r] = 1.0
            sel[25 + p % GRID, c * 128 + r] = 1.0

    ident = np.eye(128, dtype=np.float32)
    onz = np.stack([np.ones(128), np.zeros(128)], 1).astype(np.float32)
    return w_ext, w_sa, dxy2s, sel, ident, onz


def _trace(nc, tc, ctx, consts_f, need_gb):
    import concourse.mybir as mybir

    dt = mybir.dt
    AF = mybir.ActivationFunctionType
    OP = mybir.AluOpType
    bs0, bs1, ba0 = consts_f[:3]

    xT_d = nc.dram_tensor("xT", [BPC, E, T], dt.float32r, kind="ExternalInput").ap()
    wext_d = nc.dram_tensor("w_ext", [6, 128, 256], dt.float32r, kind="ExternalInput").ap()
    wsa_d = nc.dram_tensor("w_sa", [64, 4], dt.float32r, kind="ExternalInput").ap()
    dxy_d = nc.dram_tensor("dxy2s", [128, NCH * FW], dt.float32, kind="ExternalInput").ap()
    sel_d = nc.dram_tensor("sel", [FW, NCH * 128], dt.float32r, kind="ExternalInput").ap()
    id_d = nc.dram_tensor("ident", [128, 128], dt.float32r, kind="ExternalInput").ap()
    onz_d = nc.dram_tensor("onz", [128, 2], dt.float32r, kind="ExternalInput").ap()
    if need_gb:
        gb_d = nc.dram_tensor("gb", [4, 64], dt.float32, kind="ExternalInput").ap()
    out_d = nc.dram_tensor("out", [BPC, T, H], dt.float32, kind="ExternalOutput").ap()

    cpool = ctx.enter_context(tc.tile_pool(name="consts", bufs=1))
    xpool = ctx.enter_context(tc.tile_pool(name="x", bufs=12))
    wkpool = ctx.enter_context(tc.tile_pool(name="work", bufs=2))
    rpool = ctx.enter_context(tc.tile_pool(name="raw", bufs=10))
    qkpool = ctx.enter_context(tc.tile_pool(name="qk", bufs=10))
    vpool = ctx.enter_context(tc.tile_pool(name="v", bufs=10))
    apool = ctx.enter_context(tc.tile_pool(name="attn", bufs=10))
    spool = ctx.enter_context(tc.tile_pool(name="small", bufs=3))
    opool = ctx.enter_context(tc.tile_pool(name="outb", bufs=2))

    # PSUM: 1 (qkv) + 1 (transpose / sigma-alpha staging) + 6 (z/L x3) = 8
    ps_qkv = ctx.enter_context(tc.tile_pool(name="ps_qkv", bufs=1, space="PSUM"))
    ps_tp = ctx.enter_context(tc.tile_pool(name="ps_tp", bufs=1, space="PSUM"))
    ps_zl = ctx.enter_context(tc.tile_pool(name="ps_zl", bufs=3, space="PSUM"))

    w_ext = cpool.tile([128, 6 * 256], dt.float32r)
    for j in range(6):
        nc.sync.dma_start(w_ext[:, j * 256:(j + 1) * 256], wext_d[j])
    w_sa = cpool.tile([64, 4], dt.float32r)
    nc.sync.dma_start(w_sa[:], wsa_d)
    dxy2s = cpool.tile([128, NCH * FW], dt.float32)
    nc.sync.dma_start(dxy2s[:], dxy_d)
    sel = cpool.tile([FW, NCH * 128], dt.float32r)
    nc.sync.dma_start(sel[:], sel_d)
    ident = cpool.tile([128, 128], dt.float32r)
    nc.sync.dma_start(ident[:], id_d)
    onz = cpool.tile([128, 2], dt.float32r)
    nc.sync.dma_start(onz[:], onz_d)
    if need_gb:
        gb = cpool.tile([4, 64], dt.float32)
        nc.sync.dma_start(gb[:], gb_d)

    _bias_cache = {}

    def fbias(val, tsz=128):
        val = float(val)
        if val == 0.0:
            return 0.0
        if val not in _bias_cache:
            bt = cpool.tile([128, 1], dt.float32, name=f"bias{len(_bias_cache)}")
            nc.vector.memset(bt[:], val)
            _bias_cache[val] = bt
        return _bias_cache[val][0:tsz, :]

    for b in range(BPC):
        xt = []
        for j in range(6):
            xj = xpool.tile([128, T], dt.float32r, tag="xT", name=f"x{j}")
            nc.sync.dma_start(xj[:], xT_d[b, j * 128:(j + 1) * 128, :])
            xt.append(xj)

        # qkT: cols 0:578 = q^T (LN'd, * rstd/8), 578:1156 = k^T (LN'd)
        qkT = wkpool.tile([64, 1156], dt.float32r, tag="qkT")
        raw = [rpool.tile([128, 192], dt.float32, tag="raw", name=f"raw{i}")
               for i in range(NCH)]
        qk_ln = [qkpool.tile([128, 128], dt.float32r, tag="qkln", name=f"qkln{i}")
                 for i in range(NCH)]
        v_ext = [vpool.tile([128, 66], dt.float32r, tag="vext", name=f"vext{i}")
                 for i in range(NCH)]
        mv = spool.tile([128, 20], dt.float32, tag="mv")

        # ---- phase 1: QKV matmuls, raw evac, LN stats ----
        for c in range(NCH):
            tsz, toff = TSZ[c], TOFF[c]
            pqkv = ps_qkv.tile([128, 256], dt.float32, tag="qkv", name=f"qkv{c}")
            for j in range(6):
                nc.tensor.matmul(
                    pqkv[0:tsz, :], xt[j][:, toff:toff + tsz],
                    w_ext[:, j * 256:(j + 1) * 256],
                    start=(j == 0), stop=(j == 5),
                )
            nc.vector.tensor_copy(raw[c][0:tsz, :], pqkv[0:tsz, 0:192])
            st = spool.tile([128, 12], dt.float32, tag="st", name=f"st{c}")
            nc.vector.bn_stats(st[0:tsz, 0:6], raw[c][0:tsz, 0:64])
            nc.vector.bn_stats(st[0:tsz, 6:12], raw[c][0:tsz, 64:128])
            nc.vector.bn_aggr(mv[0:tsz, 4 * c:4 * c + 2], st[0:tsz, 0:6])
            nc.vector.bn_aggr(mv[0:tsz, 4 * c + 2:4 * c + 4], st[0:tsz, 6:12])

        # ---- batched LN params: scale = exp(-0.5 ln(var+eps)) [q: * 1/8] ----
        mv4 = mv[:].rearrange("p (c f) -> p c f", f=4)
        lnv = spool.tile([128, 10], dt.float32, tag="lnv")
        lnv2 = lnv[:].rearrange("p (c f) -> p c f", f=2)
        nc.scalar.activation(lnv2[:, :, :], mv4[:, :, 1::2], AF.Ln, bias=fbias(EPS))
        sc = spool.tile([128, 10], dt.float32, tag="sc")
        sc2 = sc[:].rearrange("p (c f) -> p c f", f=2)
        nc.scalar.activation(sc2[:, :, 0], lnv2[:, :, 0], AF.Exp,
                             bias=fbias(float(np.log(0.125))), scale=-0.5)
        nc.scalar.activation(sc2[:, :, 1], lnv2[:, :, 1], AF.Exp, scale=-0.5)
        nmr = spool.tile([128, 10], dt.float32, tag="nmr")
        nmr2 = nmr[:].rearrange("p (c f) -> p c f", f=2)
        nc.vector.scalar_tensor_tensor(
            nmr2[:, :, :], mv4[:, :, 0::2], -1.0, sc2[:, :, :], OP.mult, OP.mult,
        )

        # ---- phase 2: LN apply, v build, transposes ----
        for c in range(NCH):
            tsz, toff = TSZ[c], TOFF[c]
            t2 = tsz + (tsz & 1)
            if t2 != tsz:  # pre-zero the transpose pad row (base-64 access)
                nc.vector.tensor_copy(
                    qk_ln[c][64:66, :], onz[64:66, 1:2].broadcast_to([2, 128]))
            nc.vector.tensor_scalar(
                qk_ln[c][0:tsz, 0:64], raw[c][0:tsz, 0:64],
                sc[0:tsz, 2 * c:2 * c + 1], nmr[0:tsz, 2 * c:2 * c + 1],
                OP.mult, OP.add,
            )
            nc.vector.tensor_scalar(
                qk_ln[c][0:tsz, 64:128], raw[c][0:tsz, 64:128],
                sc[0:tsz, 2 * c + 1:2 * c + 2], nmr[0:tsz, 2 * c + 1:2 * c + 2],
                OP.mult, OP.add,
            )
            if need_gb:
                nc.vector.tensor_mul(qk_ln[c][0:tsz, 0:64], qk_ln[c][0:tsz, 0:64],
                                     gb[0:1, :].partition_broadcast(tsz))
                nc.vector.tensor_add(qk_ln[c][0:tsz, 0:64], qk_ln[c][0:tsz, 0:64],
                                     gb[1:2, :].partition_broadcast(tsz))
                nc.vector.tensor_mul(qk_ln[c][0:tsz, 64:128], qk_ln[c][0:tsz, 64:128],
                                     gb[2:3, :].partition_broadcast(tsz))
                nc.vector.tensor_add(qk_ln[c][0:tsz, 64:128], qk_ln[c][0:tsz, 64:128],
                                     gb[3:4, :].partition_broadcast(tsz))
            nc.vector.tensor_scalar(
                v_ext[c][0:tsz, 0:64], raw[c][0:tsz, 128:192], 1.0, None, OP.mult)
            nc.scalar.copy(v_ext[c][0:tsz, 64:66], onz[0:tsz, :])

            tpc = ps_tp.tile([128, 256], dt.float32r, tag="tp", name=f"tp{c}")
            nc.tensor.transpose(tpc[0:64, 0:t2], qk_ln[c][0:t2, 0:64],
                                ident[0:t2, 0:t2])
            nc.tensor.transpose(tpc[0:64, 128:128 + t2], qk_ln[c][0:t2, 64:128],
                                ident[0:t2, 0:t2])
            ncols = min(128, 578 - toff)
            dst = qkT[:].rearrange("p (g q) -> p g q", g=2)[:, :, toff:toff + ncols]
            nc.scalar.copy(
                dst, tpc[0:64, :].rearrange("p (g q) -> p g q", g=2)[:, :, 0:ncols])

        # ---- sigma/alpha MLP + log-space factor build ----
        psa = ps_tp.tile([128, 20], dt.float32, tag="tp")
        for c in range(NCH):
            nc.tensor.matmul(
                psa[0:TSZ[c], 4 * c:4 * c + 4], qkT[:, TOFF[c]:TOFF[c] + TSZ[c]],
                w_sa[:], start=True, stop=True,
            )
        sap = spool.tile([128, 20], dt.float32, tag="sap")
        nc.vector.tensor_copy(sap[:], psa[:, 0:20])
        sap4 = sap[:].rearrange("p (c f) -> p c f", f=4)
        texp = spool.tile([128, 10], dt.float32, tag="texp")
        texp3 = texp[:].rearrange("p (c f) -> p c f", f=2)
        if bs0 == bs1:
            nc.scalar.activation(texp3[:, :, :], sap4[:, :, 0:2], AF.Exp,
                                 bias=fbias(-bs0), scale=-1.0)
        else:
            for col in range(2):
                nc.scalar.activation(texp3[:, :, col], sap4[:, :, col], AF.Exp,
                                     bias=fbias(-(bs0 if col == 0 else bs1)),
                                     scale=-1.0)
        ab = spool.tile([128, 10], dt.float32, tag="ab")
        nc.vector.tensor_scalar_add(ab[:], texp[:], 1.0)
        nc.vector.tensor_mul(ab[:], ab[:], ab[:])
        spe = spool.tile([128, 5], dt.float32, tag="spe")
        nc.scalar.activation(spe[:], sap4[:, :, 2], AF.Exp, bias=fbias(ba0))
        spl = spool.tile([128, 5], dt.float32, tag="spl")
        nc.scalar.activation(spl[:], spe[:], AF.Ln, bias=fbias(1.0))
        lna = spool.tile([128, 5], dt.float32, tag="lna")
        nc.scalar.activation(lna[:], spl[:], AF.Ln, scale=0.125)

        yn = spool.tile([128, NCH * FW], dt.float32r, tag="yn")
        dxy3 = dxy2s[:].rearrange("p (c f) -> p c f", f=FW)
        yn3 = yn[:].rearrange("p (c f) -> p c f", f=FW)
        ab3 = ab[:].rearrange("p (c f) -> p c f", f=2)
        nc.vector.scalar_tensor_tensor(
            yn3[:, :, 0:24], dxy3[:, :, 0:24], 1.0,
            ab3[:, :, 1:2].broadcast_to([128, NCH, 24]), OP.mult, OP.mult,
        )
        nc.vector.scalar_tensor_tensor(
            yn3[:, :, 25:50], dxy3[:, :, 25:50], 1.0,
            ab3[:, :, 0:1].broadcast_to([128, NCH, 25]), OP.mult, OP.mult,
        )
        nc.vector.tensor_copy(yn3[:, :, 24:25], lna[:].unsqueeze(-1))

        yT = wkpool.tile([FW, 578], dt.float32r, tag="yT")
        for c in range(NCH):
            t2 = TSZ[c] + (TSZ[c] & 1)
            pfc = ps_tp.tile([128, 256], dt.float32r, tag="tp", name=f"pf{c}")
            nc.tensor.transpose(pfc[0:FW, 0:t2], yn[0:t2, c * FW:(c + 1) * FW],
                                ident[0:t2, 0:t2])
            nc.scalar.copy(yT[:, TOFF[c]:TOFF[c] + t2], pfc[0:FW, 0:t2])

        # ---- main loop: sim^T, bias, exp (transposed softmax) ----
        attnT = [apool.tile([128, T], dt.float32r, tag="attnT", name=f"attnT{i}")
                 for i in range(NCH)]
        for c in range(NCH):
            tsz, toff = TSZ[c], TOFF[c]
            pl = ps_zl.tile([128, 640], dt.float32, tag="zl", name=f"pl{c}")
            nc.tensor.matmul(pl[0:tsz, 0:512], sel[:, c * 128:c * 128 + tsz],
                             yT[:, 0:512], start=True, stop=True)
            nc.tensor.matmul(pl[0:tsz, 512:578], sel[:, c * 128:c * 128 + tsz],
                             yT[:, 512:578], start=True, stop=True)
            expl = wkpool.tile([128, 578], dt.float32r, tag="expl")
            nc.scalar.activation(expl[0:tsz, :], pl[0:tsz, 0:578], AF.Exp)

            kTc = qkT[:, 578 + toff:578 + toff + tsz]
            pz = ps_zl.tile([128, 640], dt.float32, tag="zl", name=f"pz{c}")
            nc.tensor.matmul(pz[0:tsz, 0:512], kTc, qkT[:, 0:512],
                             start=True, stop=False)
            nc.tensor.matmul(pz[0:tsz, 512:578], kTc, qkT[:, 512:578],
                             start=True, stop=False)
            nc.tensor.matmul(pz[0:tsz, 0:512], ident[0:tsz, 0:tsz],
                             expl[0:tsz, 0:512], start=False, stop=True)
            nc.tensor.matmul(pz[0:tsz, 512:578], ident[0:tsz, 0:tsz],
                             expl[0:tsz, 512:578], start=False, stop=True)
            nc.scalar.activation(attnT[c][0:tsz, :], pz[0:tsz, 0:T], AF.Exp)

        # ---- attn @ [v | 1], batched normalize ----
        po = ps_zl.tile([128, 640], dt.float32, tag="zl")
        for qc in range(NCH):
            qsz, qoff = TSZ[qc], TOFF[qc]
            for kc in range(NCH):
                nc.tensor.matmul(
                    po[0:qsz, 128 * qc:128 * qc + 66],
                    attnT[kc][0:TSZ[kc], qoff:qoff + qsz],
                    v_ext[kc][0:TSZ[kc], :], start=(kc == 0), stop=(kc == 4),
                )
        po5 = po[:].rearrange("p (c f) -> p c f", f=128)
        rcp = spool.tile([128, 5], dt.float32, tag="rcp")
        nc.vector.reciprocal(rcp[:], po5[:, :, 64])
        osb = opool.tile([128, 320], dt.float32, tag="osb")
        osb3 = osb[:].rearrange("p (c f) -> p c f", f=64)
        nc.vector.scalar_tensor_tensor(
            osb3[:, :, :], po5[:, :, 0:64], 1.0,
            rcp[:].unsqueeze(-1).broadcast_to([128, 5, 64]), OP.mult, OP.mult)

        nc.sync.dma_start(
            out_d[b, 0:512, :].rearrange("(c p) h -> p c h", p=128),
            osb[:, 0:256].rearrange("p (c h) -> p c h", h=64),
        )
        nc.sync.dma_start(out_d[b, 512:T, :], osb[0:65, 256:320])


_CACHE = {}


def _build(consts_f, need_gb):
    import concourse.tile as tile
    from concourse import bacc

    key = (consts_f, need_gb)
    if key in _CACHE:
        return _CACHE[key]
    nc = bacc.Bacc("TRN2", target_bir_lowering=False, debug=False)
    with tile.TileContext(nc) as tc, ExitStack() as ctx:
        _trace(nc, tc, ctx, consts_f, need_gb)
    nc.finalize()
    _CACHE[key] = nc
    return nc


def kernel(x, w_q, w_k, w_v, q_gamma, q_beta, k_gamma, k_beta,
           w_sigma, b_sigma, w_alpha, b_alpha):
    from concourse import bass_utils

    x = np.asarray(x, np.float32)
    w_q, w_k, w_v = (np.asarray(a, np.float32) for a in (w_q, w_k, w_v))
    w_sigma = np.asarray(w_sigma, np.float32)
    w_alpha = np.asarray(w_alpha, np.float32)
    b_sigma = np.asarray(b_sigma, np.float32)
    b_alpha = np.asarray(b_alpha, np.float32)
    q_gamma, q_beta = np.asarray(q_gamma, np.float32), np.asarray(q_beta, np.float32)
    k_gamma, k_beta = np.asarray(k_gamma, np.float32), np.asarray(k_beta, np.float32)

    trivial_gb = (
        np.allclose(q_gamma, 1) and np.allclose(k_gamma, 1)
        and np.allclose(q_beta, 0) and np.allclose(k_beta, 0)
    )

    w_ext, w_sa, dxy2s, sel, ident, onz = _host_consts(
        w_q, w_k, w_v, w_sigma, w_alpha)
    consts_f = (float(b_sigma[0]), float(b_sigma[1]), float(b_alpha[0]))
    nc = _build(consts_f, not trivial_gb)

    xt = np.ascontiguousarray(x.reshape(NCORES, BPC, T, E).transpose(0, 1, 3, 2))

    base = {
        "w_ext": w_ext, "w_sa": w_sa, "dxy2s": dxy2s, "sel": sel, "ident": ident,
        "onz": onz,
    }
    if not trivial_gb:
        base["gb"] = np.stack(
            [q_gamma, q_beta / 8.0, k_gamma, k_beta]).astype(np.float32)
    in_maps = [{**base, "xT": xt[c]} for c in range(NCORES)]

    res = bass_utils.run_bass_kernel_spmd(nc, in_maps, core_ids=list(range(NCORES)))
    out = np.concatenate([res.results[c]["out"] for c in range(NCORES)], axis=0)
    return out.astype(np.float32)


# revision 21
# speedup vs baseline: 1.5454x; 1.0680x over previous
"""Trainium2 Bass kernel for nn_AugmentedAttentionHead.

Math per batch b (reference):
  q = LN(x @ w_q) ; k = LN(x @ w_k) ; v = x @ w_v          [T, H]
  sim = q @ k^T ; sim[1:, 1:] += alpha * G                 [T, T]
  out = softmax(sim / 8) @ v                               [T, H]
with alpha = softplus(q[1:] @ w_alpha), sigma = sigmoid(q[1:] @ w_sigma),
G the per-query anisotropic Gaussian over the 24x24 patch grid.

Strategy: data-parallel over batch (8 batches/core on 8 cores).
Softmax is computed TRANSPOSED ([k-token partitions, q-token free]) so the
exp() output feeds the attn @ v matmul directly as the stationary operand
(contraction over k needs k on partitions) — the [T, T] attention matrix is
never transposed or evacuated.

The Gaussian bias is built in log space: per query token a [T, 50] factor
tile (24 y-terms, ln(alpha'), 24 x-terms, prefix-kill column) is built with
per-partition scalar ops, PE-transposed, and then one f32r matmul per
k-chunk against a constant 0/1 selector reconstructs
  L'[k, q] = ln(alpha'_q) - dy2(k,q)/(2 sy_q^2) - dx2(k,q)/(2 sx_q^2)
exp(L') = alpha' * G is accumulated into the sim PSUM by an identity matmul.
Prefix row/column kills are folded into the host tables (-1e30 entries).

x is passed host-pre-transposed ([E, T] per batch) so the QKV projection
contracts over E with x^T chunks stationary, producing q,k,v in natural
layout (layernorm reduces over the free axis). rstd is computed as
exp(-0.5 ln(var+eps)) so every ACT func stays in the ln/exp table set.
"""

import numpy as np
from contextlib import ExitStack

B, T, E, H = 64, 577, 768, 64
GRID = 24
EPS = 1e-5
NCORES = 8
BPC = B // NCORES

TSZ = [128, 128, 128, 128, 65]
TOFF = [0, 128, 256, 384, 512]
NCH = 5
FW = 50  # 24 y | ln-alpha | 24 x | prefix-kill
NEG = -1.0e30


def _host_consts(w_q, w_k, w_v, w_sigma, w_alpha):
    w_ext = np.zeros((6, 128, 256), np.float32)
    wqkv = np.concatenate([w_q, w_k, w_v], axis=1)
    for j in range(6):
        w_ext[j, :, :192] = wqkv[j * 128:(j + 1) * 128]

    w_sa = np.zeros((64, 4), np.float32)
    w_sa[:, 0:2] = 8.0 * w_sigma
    w_sa[:, 2] = 8.0 * w_alpha[:, 0]

    dxy2s = np.zeros((128, NCH * FW), np.float32)
    for c in range(NCH):
        for r in range(TSZ[c]):
            t = TOFF[c] + r
            o = c * FW
            dxy2s[r, o + 49] = NEG
            if t == 0:
                dxy2s[r, o:o + 24] = NEG
                dxy2s[r, o + 25:o + 49] = NEG
                continue
            p = t - 1
            py, px = p // GRID, p % GRID
            j = np.arange(GRID, dtype=np.float32)
            dxy2s[r, o:o + 24] = -0.5 * (py - j) ** 2
            dxy2s[r, o + 25:o + 49] = -0.5 * (px - j) ** 2

    sel = np.zeros((FW, NCH * 128), np.float32)
    for c in range(NCH):
        for r in range(TSZ[c]):
            t = TOFF[c] + r
            if t == 0:
                sel[49, c * 128 + r] = 1.0
                continue
            p = t - 1
            sel[p // GRID, c * 128 + r] = 1.0
            sel[24, c * 128 + r] = 1.0
            sel[25 + p % GRID, c * 128 + r] = 1.0

    ident = np.eye(128, dtype=np.float32)
    onz = np.stack([np.ones(128), np.zeros(128)], 1).astype(np.float32)
    return w_ext, w_sa, dxy2s, sel, ident, onz


def _trace(nc, tc, ctx, consts_f, need_gb):
    import concourse.mybir as mybir

    dt = mybir.dt
    AF = mybir.ActivationFunctionType
    OP = mybir.AluOpType
    bs0, bs1, ba0 = consts_f[:3]

    xT_d = nc.dram_tensor("xT", [BPC, E, T], dt.float32r, kind="ExternalInput").ap()
    wext_d = nc.dram_tensor("w_ext", [6, 128, 256], dt.float32r, kind="ExternalInput").ap()
    wsa_d = nc.dram_tensor("w_sa", [64, 4], dt.float32r, kind="ExternalInput").ap()
    dxy_d = nc.dram_tensor("dxy2s", [128, NCH * FW], dt.float32, kind="ExternalInput").ap()
    sel_d = nc.dram_tensor("sel", [FW, NCH * 128], dt.float32r, kind="ExternalInput").ap()
    id_d = nc.dram_tensor("ident", [128, 128], dt.float32r, kind="ExternalInput").ap()
    onz_d = nc.dram_tensor("onz", [128, 2], dt.float32r, kind="ExternalInput").ap()
    if need_gb:
        gb_d = nc.dram_tensor("gb", [4, 64], dt.float32, kind="ExternalInput").ap()
    out_d = nc.dram_tensor("out", [BPC, T, H], dt.float32, kind="ExternalOutput").ap()

    cpool = ctx.enter_context(tc.tile_pool(name="consts", bufs=1))
    xpool = ctx.enter_context(tc.tile_pool(name="x", bufs=12))
    wkpool = ctx.enter_context(tc.tile_pool(name="work", bufs=2))
    rpool = ctx.enter_context(tc.tile_pool(name="raw", bufs=10))
    qkpool = ctx.enter_context(tc.tile_pool(name="qk", bufs=10))
    vpool = ctx.enter_context(tc.tile_pool(name="v", bufs=10))
    apool = ctx.enter_context(tc.tile_pool(name="attn", bufs=10))
    spool = ctx.enter_context(tc.tile_pool(name="small", bufs=3))
    opool = ctx.enter_context(tc.tile_pool(name="outb", bufs=2))

    # PSUM (8 banks): qkv x2 + tp x2 + zl x4, all 1-bank slots
    ps_qkv = ctx.enter_context(tc.tile_pool(name="ps_qkv", bufs=2, space="PSUM"))
    ps_tp = ctx.enter_context(tc.tile_pool(name="ps_tp", bufs=2, space="PSUM"))
    ps_zl = ctx.enter_context(tc.tile_pool(name="ps_zl", bufs=4, space="PSUM"))

    w_ext = cpool.tile([128, 6 * 256], dt.float32r)
    for j in range(6):
        nc.sync.dma_start(w_ext[:, j * 256:(j + 1) * 256], wext_d[j])
    w_sa = cpool.tile([64, 4], dt.float32r)
    nc.sync.dma_start(w_sa[:], wsa_d)
    dxy2s = cpool.tile([128, NCH * FW], dt.float32)
    nc.sync.dma_start(dxy2s[:], dxy_d)
    sel = cpool.tile([FW, NCH * 128], dt.float32r)
    nc.sync.dma_start(sel[:], sel_d)
    ident = cpool.tile([128, 128], dt.float32r)
    nc.sync.dma_start(ident[:], id_d)
    onz = cpool.tile([128, 2], dt.float32r)
    nc.sync.dma_start(onz[:], onz_d)
    if need_gb:
        gb = cpool.tile([4, 64], dt.float32)
        nc.sync.dma_start(gb[:], gb_d)

    _bias_cache = {}

    def fbias(val, tsz=128):
        val = float(val)
        if val == 0.0:
            return 0.0
        if val not in _bias_cache:
            bt = cpool.tile([128, 1], dt.float32, name=f"bias{len(_bias_cache)}")
            nc.vector.memset(bt[:], val)
            _bias_cache[val] = bt
        return _bias_cache[val][0:tsz, :]

    for b in range(BPC):
        xt = []
        for j in range(6):
            xj = xpool.tile([128, T], dt.float32r, tag="xT", name=f"x{j}")
            nc.sync.dma_start(xj[:], xT_d[b, j * 128:(j + 1) * 128, :])
            xt.append(xj)

        # qkT: cols 0:578 = q^T (LN'd, * rstd/8), 578:1156 = k^T (LN'd)
        qkT = wkpool.tile([64, 1156], dt.float32r, tag="qkT")
        raw = [rpool.tile([128, 192], dt.float32, tag="raw", name=f"raw{i}")
               for i in range(NCH)]
        qk_ln = [qkpool.tile([128, 128], dt.float32r, tag="qkln", name=f"qkln{i}")
                 for i in range(NCH)]
        v_ext = [vpool.tile([128, 66], dt.float32r, tag="vext", name=f"vext{i}")
                 for i in range(NCH)]
        mv = spool.tile([128, 20], dt.float32, tag="mv")

        # ---- phase 1: QKV matmuls, raw evac, LN stats ----
        for c in range(NCH):
            tsz, toff = TSZ[c], TOFF[c]
            pqkv = ps_qkv.tile([128, 256], dt.float32, tag="qkv", name=f"qkv{c}")
            for j in range(6):
                nc.tensor.matmul(
                    pqkv[0:tsz, :], xt[j][:, toff:toff + tsz],
                    w_ext[:, j * 256:(j + 1) * 256],
                    start=(j == 0), stop=(j == 5),
                )
            nc.vector.tensor_copy(raw[c][0:tsz, :], pqkv[0:tsz, 0:192])
            st = spool.tile([128, 12], dt.float32, tag="st", name=f"st{c}")
            nc.vector.bn_stats(st[0:tsz, 0:6], raw[c][0:tsz, 0:64])
            nc.vector.bn_stats(st[0:tsz, 6:12], raw[c][0:tsz, 64:128])
            nc.vector.bn_aggr(mv[0:tsz, 4 * c:4 * c + 2], st[0:tsz, 0:6])
            nc.vector.bn_aggr(mv[0:tsz, 4 * c + 2:4 * c + 4], st[0:tsz, 6:12])

        # ---- batched LN params: scale = exp(-0.5 ln(var+eps)) [q: * 1/8] ----
        mv4 = mv[:].rearrange("p (c f) -> p c f", f=4)
        lnv = spool.tile([128, 10], dt.float32, tag="lnv")
        lnv2 = lnv[:].rearrange("p (c f) -> p c f", f=2)
        nc.scalar.activation(lnv2[:, :, :], mv4[:, :, 1::2], AF.Ln, bias=fbias(EPS))
        sc = spool.tile([128, 10], dt.float32, tag="sc")
        sc2 = sc[:].rearrange("p (c f) -> p c f", f=2)
        nc.scalar.activation(sc2[:, :, 0], lnv2[:, :, 0], AF.Exp,
                             bias=fbias(float(np.log(0.125))), scale=-0.5)
        nc.scalar.activation(sc2[:, :, 1], lnv2[:, :, 1], AF.Exp, scale=-0.5)
        nmr = spool.tile([128, 10], dt.float32, tag="nmr")
        nmr2 = nmr[:].rearrange("p (c f) -> p c f", f=2)
        nc.vector.scalar_tensor_tensor(
            nmr2[:, :, :], mv4[:, :, 0::2], -1.0, sc2[:, :, :], OP.mult, OP.mult,
        )

        # ---- phase 2: LN apply, v build, transposes ----
        for c in range(NCH):
            tsz, toff = TSZ[c], TOFF[c]
            t2 = tsz + (tsz & 1)
            if t2 != tsz:  # pre-zero the transpose pad row (base-64 access)
                nc.vector.tensor_copy(
                    qk_ln[c][64:66, :], onz[64:66, 1:2].broadcast_to([2, 128]))
            nc.vector.tensor_scalar(
                qk_ln[c][0:tsz, 0:64], raw[c][0:tsz, 0:64],
                sc[0:tsz, 2 * c:2 * c + 1], nmr[0:tsz, 2 * c:2 * c + 1],
                OP.mult, OP.add,
            )
            nc.vector.tensor_scalar(
                qk_ln[c][0:tsz, 64:128], raw[c][0:tsz, 64:128],
                sc[0:tsz, 2 * c + 1:2 * c + 2], nmr[0:tsz, 2 * c + 1:2 * c + 2],
                OP.mult, OP.add,
            )
            if need_gb:
                nc.vector.tensor_mul(qk_ln[c][0:tsz, 0:64], qk_ln[c][0:tsz, 0:64],
                                     gb[0:1, :].partition_broadcast(tsz))
                nc.vector.tensor_add(qk_ln[c][0:tsz, 0:64], qk_ln[c][0:tsz, 0:64],
                                     gb[1:2, :].partition_broadcast(tsz))
                nc.vector.tensor_mul(qk_ln[c][0:tsz, 64:128], qk_ln[c][0:tsz, 64:128],
                                     gb[2:3, :].partition_broadcast(tsz))
                nc.vector.tensor_add(qk_ln[c][0:tsz, 64:128], qk_ln[c][0:tsz, 64:128],
                                     gb[3:4, :].partition_broadcast(tsz))
            nc.vector.tensor_scalar(
                v_ext[c][0:tsz, 0:64], raw[c][0:tsz, 128:192], 1.0, None, OP.mult)
            nc.scalar.copy(v_ext[c][0:tsz, 64:66], onz[0:tsz, :])

            tpc = ps_tp.tile([128, 256], dt.float32r, tag="tp", name=f"tp{c}")
            nc.tensor.transpose(tpc[0:64, 0:t2], qk_ln[c][0:t2, 0:64],
                                ident[0:t2, 0:t2])
            nc.tensor.transpose(tpc[0:64, 128:128 + t2], qk_ln[c][0:t2, 64:128],
                                ident[0:t2, 0:t2])
            ncols = min(128, 578 - toff)
            dst = qkT[:].rearrange("p (g q) -> p g q", g=2)[:, :, toff:toff + ncols]
            nc.scalar.copy(
                dst, tpc[0:64, :].rearrange("p (g q) -> p g q", g=2)[:, :, 0:ncols])

        # ---- sigma/alpha MLP + log-space factor build ----
        psa = ps_tp.tile([128, 20], dt.float32, tag="tp")
        for c in range(NCH):
            nc.tensor.matmul(
                psa[0:TSZ[c], 4 * c:4 * c + 4], qkT[:, TOFF[c]:TOFF[c] + TSZ[c]],
                w_sa[:], start=True, stop=True,
            )
        sap = spool.tile([128, 20], dt.float32, tag="sap")
        nc.vector.tensor_copy(sap[:], psa[:, 0:20])
        sap4 = sap[:].rearrange("p (c f) -> p c f", f=4)
        texp = spool.tile([128, 10], dt.float32, tag="texp")
        texp3 = texp[:].rearrange("p (c f) -> p c f", f=2)
        if bs0 == bs1:
            nc.scalar.activation(texp3[:, :, :], sap4[:, :, 0:2], AF.Exp,
                                 bias=fbias(-bs0), scale=-1.0)
        else:
            for col in range(2):
                nc.scalar.activation(texp3[:, :, col], sap4[:, :, col], AF.Exp,
                                     bias=fbias(-(bs0 if col == 0 else bs1)),
                                     scale=-1.0)
        ab = spool.tile([128, 10], dt.float32, tag="ab")
        nc.vector.tensor_scalar_add(ab[:], texp[:], 1.0)
        nc.vector.tensor_mul(ab[:], ab[:], ab[:])
        spe = spool.tile([128, 5], dt.float32, tag="spe")
        nc.scalar.activation(spe[:], sap4[:, :, 2], AF.Exp, bias=fbias(ba0))
        spl = spool.tile([128, 5], dt.float32, tag="spl")
        nc.scalar.activation(spl[:], spe[:], AF.Ln, bias=fbias(1.0))
        lna = spool.tile([128, 5], dt.float32, tag="lna")
        nc.scalar.activation(lna[:], spl[:], AF.Ln, scale=0.125)

        yn = spool.tile([128, NCH * FW], dt.float32r, tag="yn")
        dxy3 = dxy2s[:].rearrange("p (c f) -> p c f", f=FW)
        yn3 = yn[:].rearrange("p (c f) -> p c f", f=FW)
        ab3 = ab[:].rearrange("p (c f) -> p c f", f=2)
        nc.vector.scalar_tensor_tensor(
            yn3[:, :, 0:24], dxy3[:, :, 0:24], 1.0,
            ab3[:, :, 1:2].broadcast_to([128, NCH, 24]), OP.mult, OP.mult,
        )
        nc.vector.scalar_tensor_tensor(
            yn3[:, :, 25:50], dxy3[:, :, 25:50], 1.0,
            ab3[:, :, 0:1].broadcast_to([128, NCH, 25]), OP.mult, OP.mult,
        )
        nc.vector.tensor_copy(yn3[:, :, 24:25], lna[:].unsqueeze(-1))

        yT = wkpool.tile([FW, 578], dt.float32r, tag="yT")
        for c in range(NCH):
            t2 = TSZ[c] + (TSZ[c] & 1)
            pfc = ps_tp.tile([128, 256], dt.float32r, tag="tp", name=f"pf{c}")
            nc.tensor.transpose(pfc[0:FW, 0:t2], yn[0:t2, c * FW:(c + 1) * FW],
                                ident[0:t2, 0:t2])
            nc.scalar.copy(yT[:, TOFF[c]:TOFF[c] + t2], pfc[0:FW, 0:t2])

        # ---- main loop: sim^T, bias, exp (transposed softmax) ----
        attnT = [apool.tile([128, T], dt.float32r, tag="attnT", name=f"attnT{i}")
                 for i in range(NCH)]
        SPL = ((0, 320), (320, 578))
        for c in range(NCH):
            tsz, toff = TSZ[c], TOFF[c]
            selc = sel[:, c * 128:c * 128 + tsz]
            kTc = qkT[:, 578 + toff:578 + toff + tsz]
            expl = wkpool.tile([128, 578], dt.float32r, tag="expl")
            for h, (lo, hi) in enumerate(SPL):
                w = hi - lo
                pl = ps_zl.tile([128, 320], dt.float32, tag="zl", name=f"pl{c}_{h}")
                nc.tensor.matmul(pl[0:tsz, 0:w], selc, yT[:, lo:hi],
                                 start=True, stop=True)
                nc.scalar.activation(expl[0:tsz, lo:hi], pl[0:tsz, 0:w], AF.Exp)
                pz = ps_zl.tile([128, 320], dt.float32, tag="zl", name=f"pz{c}_{h}")
                nc.tensor.matmul(pz[0:tsz, 0:w], kTc, qkT[:, lo:hi],
                                 start=True, stop=False)
                nc.tensor.matmul(pz[0:tsz, 0:w], ident[0:tsz, 0:tsz],
                                 expl[0:tsz, lo:hi], start=False, stop=True)
                nc.scalar.activation(attnT[c][0:tsz, lo:min(hi, T)],
                                     pz[0:tsz, 0:min(hi, T) - lo], AF.Exp)

        # ---- attn @ [v | 1], batched normalize ----
        osb = opool.tile([128, 320], dt.float32, tag="osb")
        for g, qcs in enumerate(((0, 1), (2, 3), (4,))):
            po = ps_zl.tile([128, 256], dt.float32, tag="zl", name=f"po{g}")
            for i, qc in enumerate(qcs):
                qsz, qoff = TSZ[qc], TOFF[qc]
                for kc in range(NCH):
                    nc.tensor.matmul(
                        po[0:qsz, 128 * i:128 * i + 66],
                        attnT[kc][0:TSZ[kc], qoff:qoff + qsz],
                        v_ext[kc][0:TSZ[kc], :], start=(kc == 0), stop=(kc == 4),
                    )
            n = len(qcs)
            po5 = po[:].rearrange("p (c f) -> p c f", f=128)[:, 0:n, :]
            rcp = spool.tile([128, 2], dt.float32, tag="rcp", name=f"rcp{g}")
            nc.vector.reciprocal(rcp[:, 0:n], po5[:, :, 64])
            osb3 = osb[:].rearrange("p (c f) -> p c f", f=64)[:, 2 * g:2 * g + n, :]
            nc.vector.scalar_tensor_tensor(
                osb3, po5[:, :, 0:64], 1.0,
                rcp[:, 0:n].unsqueeze(-1).broadcast_to([128, n, 64]),
                OP.mult, OP.mult)

        nc.sync.dma_start(
            out_d[b, 0:512, :].rearrange("(c p) h -> p c h", p=128),
            osb[:, 0:256].rearrange("p (c h) -> p c h", h=64),
        )
        nc.sync.dma_start(out_d[b, 512:T, :], osb[0:65, 256:320])


_CACHE = {}


def _build(consts_f, need_gb):
    import concourse.tile as tile
    from concourse import bacc

    key = (consts_f, need_gb)
    if key in _CACHE:
        return _CACHE[key]
    nc = bacc.Bacc("TRN2", target_bir_lowering=False, debug=False)
    with tile.TileContext(nc) as tc, ExitStack() as ctx:
        _trace(nc, tc, ctx, consts_f, need_gb)
    nc.finalize()
    _CACHE[key] = nc
    return nc


def kernel(x, w_q, w_k, w_v, q_gamma, q_beta, k_gamma, k_beta,
           w_sigma, b_sigma, w_alpha, b_alpha):
    from concourse import bass_utils

    x = np.asarray(x, np.float32)
    w_q, w_k, w_v = (np.asarray(a, np.float32) for a in (w_q, w_k, w_v))
    w_sigma = np.asarray(w_sigma, np.float32)
    w_alpha = np.asarray(w_alpha, np.float32)
    b_sigma = np.asarray(b_sigma, np.float32)
    b_alpha = np.asarray(b_alpha, np.float32)
    q_gamma, q_beta = np.asarray(q_gamma, np.float32), np.asarray(q_beta, np.float32)
    k_gamma, k_beta = np.asarray(k_gamma, np.float32), np.asarray(k_beta, np.float32)

    trivial_gb = (
        np.allclose(q_gamma, 1) and np.allclose(k_gamma, 1)
        and np.allclose(q_beta, 0) and np.allclose(k_beta, 0)
    )

    w_ext, w_sa, dxy2s, sel, ident, onz = _host_consts(
        w_q, w_k, w_v, w_sigma, w_alpha)
    consts_f = (float(b_sigma[0]), float(b_sigma[1]), float(b_alpha[0]))
    nc = _build(consts_f, not trivial_gb)

    xt = np.ascontiguousarray(x.reshape(NCORES, BPC, T, E).transpose(0, 1, 3, 2))

    base = {
        "w_ext": w_ext, "w_sa": w_sa, "dxy2s": dxy2s, "sel": sel, "ident": ident,
        "onz": onz,
    }
    if not trivial_gb:
        base["gb"] = np.stack(
            [q_gamma, q_beta / 8.0, k_gamma, k_beta]).astype(np.float32)
    in_maps = [{**base, "xT": xt[c]} for c in range(NCORES)]

    res = bass_utils.run_bass_kernel_spmd(nc, in_maps, core_ids=list(range(NCORES)))
    out = np.concatenate([res.results[c]["out"] for c in range(NCORES)], axis=0)
    return out.astype(np.float32)
